# revision 8
# baseline (speedup 1.0000x reference)
"""Trainium2 Bass kernel for BaseBertSelfAttention (B=2, S=2048, H=1024, 16 heads).

Sharding (8 NeuronCores):
  - Tensor-parallel on heads: core c owns heads (2c, 2c+1) -> d_local = 128.
  - Each core: QKV projections (column-parallel) for its 2 heads over BOTH
    batches, attention in transposed layout (scores^T: keys on partitions,
    queries on the free axis), softmax denominator via a ones-augmented V
    column, normalized context ctx^T [d_local=128, B*S].
  - One 8-rank AllToAll redistributes ctx^T from head-sharding to
    row-sharding (1 MB/core, bf16).
  - Each core then computes Wo projection + residual + LayerNorm for its 512
    output rows (row j of 8 chunks of the flattened [B*S, H] output).

Precision: bf16 matmul inputs (4x PE throughput), fp32 PSUM accumulation,
fp32 softmax denominators / reciprocal / residual / LayerNorm.  The final
output is dominated by the fp32 residual + LayerNorm path, so end-to-end
relative error stays ~1e-4.
"""

import numpy as np
import ml_dtypes

import concourse.bass as bass
import concourse.tile as tile
from concourse import bacc, mybir
from concourse.bass_utils import run_bass_kernel_spmd

BF16 = mybir.dt.bfloat16
F32 = mybir.dt.float32
AF = mybir.ActivationFunctionType
P = 128

B, S, H = 2, 2048, 1024
NH, HD = 16, 64
NCORES = 8
EPS = 1e-12
SCALE = 1.0 / 8.0  # 1/sqrt(HD)

_CACHE: dict = {}


def _build_program(s=S):
    """Build the (identical-across-cores) Bass program."""
    nkb = s // P               # key blocks of 128
    qc_per_b = NCORES // B     # q chunks per batch (4)
    rows = (B * s) // NCORES   # output rows per core / q-chunk width (512)
    qw = rows
    ho = H // P                # h chunks of 128 (8)

    nc = bacc.Bacc("TRN2", target_bir_lowering=False, debug=False,
                   num_devices=NCORES)
    xT = nc.dram_tensor("xT", [B, H, s], BF16, kind="ExternalInput")
    wq = nc.dram_tensor("wq", [H, P], BF16, kind="ExternalInput")
    wk = nc.dram_tensor("wk", [H, P], BF16, kind="ExternalInput")
    wv = nc.dram_tensor("wv", [H, P], BF16, kind="ExternalInput")
    wo = nc.dram_tensor("wo", [H, H], BF16, kind="ExternalInput")
    bq = nc.dram_tensor("bq", [P], F32, kind="ExternalInput")
    bk = nc.dram_tensor("bk", [P], F32, kind="ExternalInput")
    bv = nc.dram_tensor("bv", [P], F32, kind="ExternalInput")
    maskT = nc.dram_tensor("maskT", [B, P, nkb], F32, kind="ExternalInput")
    xres = nc.dram_tensor("xres", [rows, H], F32, kind="ExternalInput")
    gamma = nc.dram_tensor("gamma", [H], F32, kind="ExternalInput")
    beta = nc.dram_tensor("beta", [H], F32, kind="ExternalInput")
    out = nc.dram_tensor("out", [rows, H], F32, kind="ExternalOutput")

    with tile.TileContext(nc) as tc:
        _kernel_body(
            tc, s, nkb, qw, qc_per_b, rows, ho,
            xT, wq, wk, wv, wo, bq, bk, bv, maskT, xres, gamma, beta, out,
        )
    nc.compile()
    return nc


def _kernel_body(tc, s, nkb, qw, qc_per_b, rows, ho,
                 xT, wq, wk, wv, wo, bq, bk, bv, maskT, xres, gamma, beta, out):
    nc = tc.nc
    VPAD = 80  # padded free width of the ones-augmented V tiles (65 used)

    import contextlib
    stack = contextlib.ExitStack()
    with stack:
        consts = stack.enter_context(tc.tile_pool(name="consts", bufs=1))
        dram = stack.enter_context(tc.tile_pool(name="dram", bufs=1, space="DRAM"))

        # ---------------- constant / input loads ----------------
        wq_sb = consts.tile([P, ho, P], BF16)
        wk_sb = consts.tile([P, ho, P], BF16)
        wv_sb = consts.tile([P, ho, P], BF16)
        nc.sync.dma_start(wq_sb, wq.rearrange("(o p) d -> p o d", p=P))
        nc.sync.dma_start(wk_sb, wk.rearrange("(o p) d -> p o d", p=P))
        nc.sync.dma_start(wv_sb, wv.rearrange("(o p) d -> p o d", p=P))
        wo_sb = consts.tile([P, ho, H], BF16)
        nc.sync.dma_start(wo_sb, wo.rearrange("(o p) n -> p o n", p=P))

        bq_sb = consts.tile([P, 1], F32)
        bk_sb = consts.tile([P, 1], F32)
        nc.sync.dma_start(bq_sb, bq[:, None])
        nc.sync.dma_start(bk_sb, bk[:, None])
        # bv broadcast across partitions: [P, 128] where every row is bv
        bv_b = consts.tile([P, P], F32)
        bv_ap = bv[:]
        nc.gpsimd.dma_start(
            bv_b, bass.AP(tensor=bv_ap.tensor, offset=bv_ap.offset,
                          ap=[[0, P], *bv_ap.ap]))
        gamma_b = consts.tile([P, H], F32)
        g_ap = gamma[:]
        nc.gpsimd.dma_start(
            gamma_b, bass.AP(tensor=g_ap.tensor, offset=g_ap.offset,
                             ap=[[0, P], *g_ap.ap]))
        beta_b = consts.tile([P, H], F32)
        b_ap = beta[:]
        nc.gpsimd.dma_start(
            beta_b, bass.AP(tensor=b_ap.tensor, offset=b_ap.offset,
                            ap=[[0, P], *b_ap.ap]))

        mask_sb = consts.tile([P, B, nkb], F32)
        nc.sync.dma_start(mask_sb, maskT.rearrange("b p k -> p b k"))

        ones_sb = consts.tile([P, P], BF16)
        nc.vector.memset(ones_sb, 1.0)
        eps_sb = consts.tile([P, 1], F32)
        nc.vector.memset(eps_sb, EPS)

        # x^T (bf16): [p(h-inner), b, h-outer, s]; load per (b, o) for pipelining
        xT_sb = consts.tile([P, B, ho, s], BF16)
        xT_r = xT.rearrange("b (o p) s -> p b o s", p=P)
        for b in range(B):
            for o in range(ho):
                nc.sync.dma_start(xT_sb[:, b, o, :], xT_r[:, b, o, :])

        xres_sb = consts.tile([P, rows // P, H], F32)
        nc.sync.dma_start(xres_sb, xres.rearrange("(r p) h -> p r h", p=P))

        # attention intermediates
        qT_sb = consts.tile([P, B, s], BF16)   # Q^T [d_local, b, s]
        kT_sb = consts.tile([P, B, s], BF16)   # K^T [d_local, b, s]
        # ones-augmented V (natural layout), per head: [p(s-inner), b, kb, 65]
        v_e = consts.tile([P, B, nkb, VPAD], BF16)
        v_o = consts.tile([P, B, nkb, VPAD], BF16)
        nc.vector.memset(v_e, 1.0)
        nc.vector.memset(v_o, 1.0)

        # A2A bounce buffers (DRAM, local)
        a2a_in = dram.tile([NCORES * P, qw], BF16)
        a2a_out = dram.tile([NCORES * P, qw], BF16)

        # ---------------- stage 1: QKV projections ----------------
        with tc.tile_pool(name="ps_qkv", bufs=2, space="PSUM") as ps_qkv:
            for b in range(B):
                for sc in range(s // 512):
                    sl = slice(sc * 512, (sc + 1) * 512)
                    for w_sb, bias_sb, dst in (
                        (wq_sb, bq_sb, qT_sb),
                        (wk_sb, bk_sb, kT_sb),
                    ):
                        ps = ps_qkv.tile([P, 512], F32, tag="qk")
                        for o in range(ho):
                            nc.tensor.matmul(
                                ps, lhsT=w_sb[:, o, :], rhs=xT_sb[:, b, o, sl],
                                start=(o == 0), stop=(o == ho - 1))
                        nc.vector.tensor_tensor(
                            dst[:, b, sl], ps,
                            bias_sb[:, 0:1].to_broadcast((P, 512)),
                            mybir.AluOpType.add)
                for kb in range(nkb):
                    ksl = slice(kb * P, (kb + 1) * P)
                    ps = ps_qkv.tile([P, P], F32, tag="v")
                    for o in range(ho):
                        nc.tensor.matmul(
                            ps, lhsT=xT_sb[:, b, o, ksl], rhs=wv_sb[:, o, :],
                            start=(o == 0), stop=(o == ho - 1))
                    nc.vector.tensor_tensor(
                        v_e[:, b, kb, 0:64], ps[:, 0:64], bv_b[:, 0:64],
                        mybir.AluOpType.add)
                    nc.vector.tensor_tensor(
                        v_o[:, b, kb, 0:64], ps[:, 64:128], bv_b[:, 64:128],
                        mybir.AluOpType.add)

        # ---------------- stage 2: attention ----------------
        with tc.tile_pool(name="ps_s", bufs=2, space="PSUM") as ps_s, \
             tc.tile_pool(name="ps_ctx", bufs=1, space="PSUM") as ps_ctx, \
             tc.tile_pool(name="ps_e", bufs=1, space="PSUM") as ps_e, \
             tc.tile_pool(name="ptile", bufs=3) as ptile, \
             tc.tile_pool(name="misc", bufs=2) as misc:
            for b in range(B):
                for qc in range(qc_per_b):
                    qsl = slice(qc * qw, (qc + 1) * qw)
                    shard = b * qc_per_b + qc
                    ctx_e = ps_ctx.tile([P, qw], F32, tag="ce")
                    ctx_o = ps_ctx.tile([P, qw], F32, tag="co")
                    for kb in range(nkb):
                        ksl = slice(kb * P, (kb + 1) * P)
                        sp = ps_s.tile([P, 2, qw], F32, tag="s")
                        nc.tensor.matmul(
                            sp[:, 0, :], lhsT=kT_sb[0:64, b, ksl],
                            rhs=qT_sb[0:64, b, qsl], start=True, stop=True)
                        nc.tensor.matmul(
                            sp[:, 1, :], lhsT=kT_sb[64:128, b, ksl],
                            rhs=qT_sb[64:128, b, qsl], start=True, stop=True)
                        pp = ptile.tile([P, 2, qw], BF16, tag="p")
                        nc.scalar.activation(
                            pp, sp, AF.Exp,
                            bias=mask_sb[:, b, kb:kb + 1], scale=SCALE)
                        nc.tensor.matmul(
                            ctx_e[0:65, :], lhsT=v_e[:, b, kb, 0:65],
                            rhs=pp[:, 0, :], start=(kb == 0), stop=(kb == nkb - 1),
                            skip_group_check=True)
                        nc.tensor.matmul(
                            ctx_o[0:65, :], lhsT=v_o[:, b, kb, 0:65],
                            rhs=pp[:, 1, :], start=(kb == 0), stop=(kb == nkb - 1),
                            skip_group_check=True)
                    # normalize: ctx[d, q] / denom[q]  (denom = row 64)
                    for h, ctx_ps in enumerate((ctx_e, ctx_o)):
                        rinv = misc.tile([1, qw], F32, tag="rinv")
                        nc.vector.reciprocal(rinv, ctx_ps[64:65, :])
                        rb = misc.tile([1, qw], BF16, tag="rb")
                        nc.vector.tensor_copy(out=rb, in_=rinv)
                        eb = ps_e.tile([64, qw], F32, tag="e")
                        nc.tensor.matmul(
                            eb, lhsT=ones_sb[0:1, 0:64], rhs=rb,
                            start=True, stop=True)
                        ctx_bf = misc.tile([64, qw], BF16, tag="cb")
                        nc.vector.tensor_copy(out=ctx_bf, in_=ctx_ps[0:64, :])
                        cn = misc.tile([64, qw], BF16, tag="cn")
                        nc.vector.tensor_tensor(
                            cn, ctx_bf, eb, mybir.AluOpType.mult)
                        r0 = shard * P + h * 64
                        nc.sync.dma_start(a2a_in[r0:r0 + 64, :], cn)

        # ---------------- stage 3: AllToAll ----------------
        nc.gpsimd.collective_compute(
            "AllToAll", mybir.AluOpType.bypass,
            replica_groups=[list(range(NCORES))],
            ins=[a2a_in[:].opt()], outs=[a2a_out[:].opt()])

        # ---------------- stage 4: Wo + residual + LayerNorm ----------------
        ctxf = consts.tile([P, ho, qw], BF16)
        nc.sync.dma_start(ctxf, a2a_out.rearrange("(o p) q -> p o q", p=P))
        with tc.tile_pool(name="ps_o", bufs=2, space="PSUM") as ps_o, \
             tc.tile_pool(name="fin", bufs=2) as fin:
            for qt in range(rows // P):
                res = fin.tile([P, H], F32, tag="res")
                for nch in range(H // 512):
                    nsl = slice(nch * 512, (nch + 1) * 512)
                    ps = ps_o.tile([P, 512], F32, tag="o")
                    for o in range(ho):
                        nc.tensor.matmul(
                            ps, lhsT=ctxf[:, o, qt * P:(qt + 1) * P],
                            rhs=wo_sb[:, o, nsl],
                            start=(o == 0), stop=(o == ho - 1))
                    nc.vector.tensor_tensor(
                        res[:, nsl], ps, xres_sb[:, qt, nsl],
                        mybir.AluOpType.add)
                # LayerNorm over H (free axis)
                stats = fin.tile([P, H // 512, 6], F32, tag="st")
                for g in range(H // 512):
                    nc.vector.bn_stats(
                        stats[:, g, :], res[:, g * 512:(g + 1) * 512])
                mv = fin.tile([P, 2], F32, tag="mv")
                nc.vector.bn_aggr(out=mv, in_=stats)
                rstd = fin.tile([P, 1], F32, tag="rstd")
                nc.scalar.activation(rstd, mv[:, 1:2], AF.Sqrt, bias=eps_sb)
                nc.vector.reciprocal(rstd, rstd)
                nc.vector.tensor_tensor(
                    res, res, mv[:, 0:1].to_broadcast((P, H)),
                    mybir.AluOpType.subtract)
                nc.vector.tensor_tensor(
                    res, res, rstd[:, 0:1].to_broadcast((P, H)),
                    mybir.AluOpType.mult)
                outt = fin.tile([P, H], F32, tag="outt")
                nc.vector.tensor_tensor(outt, res, gamma_b, mybir.AluOpType.mult)
                nc.vector.tensor_tensor(outt, outt, beta_b, mybir.AluOpType.add)
                nc.sync.dma_start(out[qt * P:(qt + 1) * P, :], outt)


def get_program(s=S):
    key = ("nc", s)
    if key not in _CACHE:
        _CACHE[key] = _build_program(s)
    return _CACHE[key]


def make_in_maps(hidden_states, attention_mask, Wq, bq, Wk, bk, Wv, bv, Wo, bo,
                 ln_gamma, ln_beta):
    """Host-side sharding: build the 8 per-core input maps."""
    bf = ml_dtypes.bfloat16
    hs = np.asarray(hidden_states, dtype=np.float32)
    b_, s_, h_ = hs.shape
    nkb = s_ // P
    rows = (b_ * s_) // NCORES
    qc_per_b = NCORES // b_

    xT = np.ascontiguousarray(hs.transpose(0, 2, 1)).astype(bf)  # [B, H, S]
    Wq = np.asarray(Wq, np.float32)
    Wk = np.asarray(Wk, np.float32)
    Wv = np.asarray(Wv, np.float32)
    wo_bf = np.ascontiguousarray(np.asarray(Wo, np.float32)).astype(bf)
    bq = np.asarray(bq, np.float32)
    bk = np.asarray(bk, np.float32)
    bv = np.asarray(bv, np.float32)
    bo = np.asarray(bo, np.float32)
    gamma = np.ascontiguousarray(np.asarray(ln_gamma, np.float32))
    beta = np.ascontiguousarray(np.asarray(ln_beta, np.float32))
    mask = np.asarray(attention_mask, np.float32).reshape(b_, s_)
    maskT = np.ascontiguousarray(
        mask.reshape(b_, nkb, P).transpose(0, 2, 1))  # [B, P, nkb]

    in_maps = []
    for c in range(NCORES):
        d0 = c * P
        b_out, j = divmod(c, qc_per_b)
        rsl = slice(j * rows, (j + 1) * rows)
        in_maps.append({
            "xT": xT,
            "wq": np.ascontiguousarray(Wq[:, d0:d0 + P]).astype(bf),
            "wk": np.ascontiguousarray(Wk[:, d0:d0 + P]).astype(bf),
            "wv": np.ascontiguousarray(Wv[:, d0:d0 + P]).astype(bf),
            "wo": wo_bf,
            "bq": np.ascontiguousarray(bq[d0:d0 + P]),
            "bk": np.ascontiguousarray(bk[d0:d0 + P]),
            "bv": np.ascontiguousarray(bv[d0:d0 + P]),
            "maskT": maskT,
            "xres": np.ascontiguousarray(hs[b_out, rsl, :] + bo[None, :]),
            "gamma": gamma,
            "beta": beta,
        })
    return in_maps


def assemble_output(results, b_=B, s_=S, h_=H):
    rows = (b_ * s_) // NCORES
    qc_per_b = NCORES // b_
    out = np.empty((b_, s_, h_), np.float32)
    for c in range(NCORES):
        b_out, j = divmod(c, qc_per_b)
        out[b_out, j * rows:(j + 1) * rows, :] = np.asarray(
            results[c]["out"], np.float32)
    return out


def kernel(**inputs):
    nc = get_program(S)
    in_maps = make_in_maps(**inputs)
    res = run_bass_kernel_spmd(nc, in_maps, list(range(NCORES)))
    return assemble_output(res.results)
